# revision 7
# baseline (speedup 1.0000x reference)
"""Block 8x8 2D-IDCT kernel for Trainium2 (Bass/Tile), 8-core data-parallel.

Full input x_dct (4,64,64,64,8,8) f32 is sharded along flattened (N,C) into
8 shards of 32 images; each core independently computes the 2D IDCT of its
32 images and writes (32,512,512); results are concatenated on host.

Per-core pipeline (per 1MiB image tile):
  DMA load [128p x 2048] (8KB/partition contiguous)
  -> PE transpose 16x [128,128] sub-tiles (block-pair coeffs -> partitions)
  -> DVE copy PSUM->SBUF
  -> fp32 matmul: lhsT = transposed data (stationary), rhs = blockdiag(G^T,G^T),
     G = kron(M,M); output [pair, 128 pixels of 2 blocks] in PSUM
  -> ACT copy PSUM->SBUF with permuted AP (store-contiguous free dim)
  -> DMA store 1MiB with 1KiB contiguous DRAM runs
"""

import math
from contextlib import ExitStack

import numpy as np

import concourse.bass as bass
import concourse.mybir as mybir
import concourse.tile as tile
from concourse import bacc, masks
from concourse.bass_utils import run_bass_kernel_spmd

F32 = mybir.dt.float32

N_CORES = 8
IMGS = 32          # images per core
P = 128            # partitions
SUBT = 16          # [128,128] sub-tiles per image tile
GRPS = 4           # sub-tiles per PSUM bank group
BLOCK = 8


def _make_idct_matrix(nb: int) -> np.ndarray:
    m = np.zeros((nb, nb), dtype=np.float64)
    for n in range(nb):
        for k in range(nb):
            alpha = math.sqrt(1.0 / nb) if k == 0 else math.sqrt(2.0 / nb)
            m[n, k] = alpha * math.cos(math.pi * (2 * n + 1) * k / (2 * nb))
    return m.astype(np.float32)


def _build_nc(imgs: int = IMGS) -> bass.Bass:
    nc = bacc.Bacc("TRN2", target_bir_lowering=False, debug=False)

    x = nc.dram_tensor("x", [imgs, P, 2048], F32, kind="ExternalInput")
    g2 = nc.dram_tensor("g2", [P, P], F32, kind="ExternalInput")
    out = nc.dram_tensor("out", [imgs, 512, 512], F32, kind="ExternalOutput")

    with tile.TileContext(nc) as tc, ExitStack() as ctx:
        consts = ctx.enter_context(tc.tile_pool(name="consts", bufs=1))
        lpool = ctx.enter_context(tc.tile_pool(name="load", bufs=3))
        s1pool = ctx.enter_context(tc.tile_pool(name="s1", bufs=3))
        s3pool = ctx.enter_context(tc.tile_pool(name="s3", bufs=3))
        pt = ctx.enter_context(
            tc.tile_pool(name="pt", bufs=2, space=bass.MemorySpace.PSUM)
        )
        po = ctx.enter_context(
            tc.tile_pool(name="po", bufs=2, space=bass.MemorySpace.PSUM)
        )

        ident = consts.tile([P, P], F32)
        masks.make_identity(nc, ident[:])
        g2t = consts.tile([P, P], F32)
        nc.sync.dma_start(g2t[:], g2[:])

        for img in range(imgs):
            L = lpool.tile([P, 2048], F32)
            nc.sync.dma_start(L[:], x[:][img])
            S3 = s3pool.tile([P, 2048], F32)
            # S3 free layout: i*256 + s*16 + g*8 + j  (s = grp*4 + d, dg = d*2+g)
            s3v = S3[:].rearrange(
                "p (i grp dg j) -> p grp dg i j", i=8, grp=GRPS, dg=8, j=8
            )
            for grp in range(GRPS):
                T1 = pt.tile([P, 512], F32)
                S1 = s1pool.tile([P, 512], F32)
                O2 = po.tile([P, 512], F32)
                for d in range(4):
                    s = grp * 4 + d
                    nc.tensor.transpose(
                        T1[:, d * P : (d + 1) * P],
                        L[:, s * P : (s + 1) * P],
                        ident[:],
                    )
                nc.vector.tensor_copy(S1[:], T1[:])
                for d in range(4):
                    nc.tensor.matmul(
                        O2[:, d * P : (d + 1) * P],
                        S1[:, d * P : (d + 1) * P],
                        g2t[:],
                        start=True,
                        stop=True,
                    )
                o2v = O2[:].rearrange("p (dg i j) -> p dg i j", dg=8, i=8, j=8)
                nc.scalar.copy(s3v[:, grp], o2v)
            # store: DRAM (u i) (half sgj); SBUF partition (u half), free
            # (i sgj) contiguous. One DMA per i keeps APs at 3 dims.
            oimg = out[:][img].rearrange(
                "(u i) (half sgj) -> u i half sgj", u=64, i=8, half=2, sgj=256
            )
            for i in range(8):
                # SBUF partition order q = u*2+half matches DRAM (u, half)
                # enumeration, so the SBUF side stays a plain 2-dim slice.
                nc.sync.dma_start(oimg[:, i], S3[:, i * 256 : (i + 1) * 256])

    nc.finalize()
    return nc


def _g2_matrix(idct_mat: np.ndarray) -> np.ndarray:
    m = np.asarray(idct_mat, dtype=np.float32)
    g = np.kron(m, m)  # g[(i,j),(k,m)] = M[i,k] * M[j,m]
    g2 = np.zeros((P, P), dtype=np.float32)
    g2[:64, :64] = g.T
    g2[64:, 64:] = g.T
    return g2


def _run(x_dct, idct_mat, H, W, trace: bool = False):
    x = np.ascontiguousarray(np.asarray(x_dct, dtype=np.float32))
    assert x.shape == (4, 64, 64, 64, BLOCK, BLOCK), x.shape
    H = int(H)
    W = int(W)
    assert H == 512 and W == 512, (H, W)

    g2 = _g2_matrix(idct_mat)
    xs = x.reshape(N_CORES, IMGS, P, 2048)

    nc = _build_nc(IMGS)
    in_maps = [{"x": xs[c], "g2": g2} for c in range(N_CORES)]
    res = run_bass_kernel_spmd(
        nc, in_maps, core_ids=list(range(N_CORES)), trace=trace
    )
    outs = [res.results[c]["out"] for c in range(N_CORES)]
    full = np.concatenate(outs, axis=0).reshape(4, 64, 512, 512)
    return full[:, :, :H, :W], res


def kernel(x_dct, idct_mat=None, H=512, W=512):
    if idct_mat is None:
        idct_mat = _make_idct_matrix(BLOCK)
    out, _ = _run(x_dct, idct_mat, H, W, trace=False)
    return out
